# revision 1
# baseline (speedup 1.0000x reference)
"""Contrastive-loss kernel for 8 Trainium2 NeuronCores.

Math (reference):
    sim = X @ X.T                               # [n, n]
    pos = targets[:,None] == targets[None,:]
    loss = ( sum(where(pos & sim<1,  1-sim, 0))
           + sum(where(~pos & sim>m, sim,  0)) ) / n    with m = 0.3

Device decomposition (per element s of sim, with a = relu(1-s),
u = relu(s-m), c = m*step(s-m), z = a - u - c):
    f_neg(s) = s*step(s-m) = u + c
    f_pos(s) = a
    total = sum_all(u) + sum_all(c) + sum_pos(z)
The diagonal (i==j) is a "pos" pair and cancels exactly: z + u + c = a = 0
for s ~ ||x||^2 >> 1.

Sharding: data-parallel over rows. Core r computes the [8192, 1024] block
T[j, i] = <x_j, x_i> for its 1024 local columns i, as 64 j-tiles of
[128, 1024] via bf16 PE matmuls (K=512 contracted in 4 chunks of 128) from
a host-pretransposed XT = X.T.  sum_pos(z) is evaluated without ever
materializing the [n, n] label mask: per j-tile the PE also computes
P_j.T @ z into a persistent PSUM accumulator ([128 classes, 1024 i],
accumulated over all 64 j-tiles), which at the end is reduced against
P_local.T (one-hot of local labels) on the DVE.  sum(u) / sum(c) row-sums
ride for free on the ACT / DVE ops that produce u and c.

Host does: transpose + bf16 cast of X, one-hot of targets, final sum of
8 x [128, 3] partials.
"""

import numpy as np
import ml_dtypes

N = 8192
D = 512
C = 128          # number of classes
NCORES = 8
NL = N // NCORES  # local columns per core (1024)
KT = D // 128     # k tiles (4)
NT = N // 128     # j tiles (64)
NCHUNK = 4        # xt free-dim chunks
CHW = N // NCHUNK  # chunk width (2048)
JT_PER_CHUNK = NT // NCHUNK  # 16
MARGIN = 0.3

_BF16 = ml_dtypes.bfloat16

_COMPILED = None     # cached (nc,) so repeat kernel() calls skip rebuild
LAST_RESULTS = None  # BassKernelResults of the most recent run (for profiling)


def _build():
    import concourse.tile as tile
    from concourse import bacc, mybir

    nc = bacc.Bacc("TRN2", target_bir_lowering=False, debug=False,
                   num_devices=NCORES)
    bf16 = mybir.dt.bfloat16
    f32 = mybir.dt.float32

    xt_d = nc.dram_tensor("xt", [D, N], bf16, kind="ExternalInput").ap()
    xtl_d = nc.dram_tensor("xt_loc", [D, NL], bf16, kind="ExternalInput").ap()
    p_d = nc.dram_tensor("p", [N, C], bf16, kind="ExternalInput").ap()
    p3_d = nc.dram_tensor("p3", [N, C], bf16, kind="ExternalInput").ap()
    plt_d = nc.dram_tensor("ploc_t", [C, NL], bf16, kind="ExternalInput").ap()
    out_d = nc.dram_tensor("out", [128, 4], f32, kind="ExternalOutput").ap()

    with tile.TileContext(nc) as tc:
        with (
            tc.tile_pool(name="xt", bufs=1) as xt_pool,
            tc.tile_pool(name="xtl", bufs=1) as xtl_pool,
            tc.tile_pool(name="pp", bufs=1) as p_pool,
            tc.tile_pool(name="acc", bufs=1) as acc_pool,
            tc.tile_pool(name="work", bufs=4) as work,
            tc.tile_pool(name="psum_s", bufs=3, space="PSUM") as psum_s_pool,
            tc.tile_pool(name="psum_p", bufs=1, space="PSUM") as psum_p_pool,
        ):
            # -- resident inputs ------------------------------------------
            xtl_sb = []
            for kt in range(KT):
                t = xtl_pool.tile([128, NL], bf16, tag=f"xtl{kt}")
                nc.sync.dma_start(t[:], xtl_d[kt * 128:(kt + 1) * 128, :])
                xtl_sb.append(t)

            # xt chunk 0 right after xt_loc so PE can start ASAP; the 4MB of
            # p/p3 (first needed a few us in) go after it, then chunks 1-3
            xt_sb = [[None] * NCHUNK for _ in range(KT)]
            for ch in range(NCHUNK):
                for kt in range(KT):
                    xt_tile = xt_pool.tile([128, CHW], bf16,
                                           tag=f"xt{kt}_{ch}")
                    xt_sb[kt][ch] = xt_tile

            def load_xt_chunk(ch):
                for kt in range(KT):
                    nc.sync.dma_start(
                        xt_sb[kt][ch][:],
                        xt_d[kt * 128:(kt + 1) * 128,
                             ch * CHW:(ch + 1) * CHW],
                    )

            load_xt_chunk(0)

            p_sb = p_pool.tile([128, NT, C], bf16)
            p_view = p_d.rearrange("(t p) c -> p t c", p=128)
            # p3 = -bf16(0.3) * P, merges the 0.3*step correction into the
            # same PSUM accumulator as the z2 projection
            p3_sb = p_pool.tile([128, NT, C], bf16)
            p3_view = p3_d.rearrange("(t p) c -> p t c", p=128)
            for tch in range(8):
                nc.sync.dma_start(
                    p_sb[:, tch * 8:(tch + 1) * 8, :],
                    p_view[:, tch * 8:(tch + 1) * 8, :],
                )
                nc.sync.dma_start(
                    p3_sb[:, tch * 8:(tch + 1) * 8, :],
                    p3_view[:, tch * 8:(tch + 1) * 8, :],
                )

            for ch in range(1, NCHUNK):
                load_xt_chunk(ch)

            plt_sb = acc_pool.tile([C, NL], bf16)
            nc.sync.dma_start(plt_sb[:], plt_d[:])

            # -- persistent accumulators ----------------------------------
            accu = acc_pool.tile([128, NT], f32)   # per-j-tile row sums of u
            accc = acc_pool.tile([128, NT], f32)   # per-j-tile counts of c
            # accumulates sum_j (P_j.T @ z2 - 0.30078125 * P_j.T @ c)
            psum_projz = psum_p_pool.tile([128, NL], f32)

            bias_m = acc_pool.tile([128, 1], f32)  # ACT bias for relu(s - m)
            nc.vector.memset(bias_m[:], -MARGIN)

            relu = mybir.ActivationFunctionType.Relu
            alu = mybir.AluOpType

            def emit_proj(jt, z_sb, c_sb):
                for h in range(2):
                    nc.tensor.matmul(
                        psum_projz[:, h * 512:(h + 1) * 512],
                        lhsT=p_sb[:, jt, :],
                        rhs=z_sb[:, h * 512:(h + 1) * 512],
                        start=(jt == 0),
                        stop=False,
                    )
                    nc.tensor.matmul(
                        psum_projz[:, h * 512:(h + 1) * 512],
                        lhsT=p3_sb[:, jt, :],
                        rhs=c_sb[:, h * 512:(h + 1) * 512],
                        start=False,
                        stop=(jt == NT - 1),
                    )

            pending = None  # (jt, z_sb, c_sb) — proj deferred one tile so
            # PE never stalls waiting on the DVE outputs of the same tile
            for jt in range(NT):
                ch, off = jt // JT_PER_CHUNK, (jt % JT_PER_CHUNK) * 128

                # s tile: [128 j, 1024 i] f32 in PSUM
                psum_s = psum_s_pool.tile([128, NL], f32, tag="psum_s")
                for h in range(2):
                    for kt in range(KT):
                        nc.tensor.matmul(
                            psum_s[:, h * 512:(h + 1) * 512],
                            lhsT=xt_sb[kt][ch][:, off:off + 128],
                            rhs=xtl_sb[kt][:, h * 512:(h + 1) * 512],
                            start=(kt == 0),
                            stop=(kt == KT - 1),
                        )

                if pending is not None:
                    emit_proj(*pending)

                a_sb = work.tile([128, NL], bf16, tag="a")
                nc.scalar.activation(a_sb[:], psum_s[:], relu,
                                     bias=1.0, scale=-1.0)
                u_sb = work.tile([128, NL], bf16, tag="u")
                nc.scalar.activation(u_sb[:], psum_s[:], relu,
                                     bias=bias_m[:], scale=1.0,
                                     accum_out=accu[:, jt:jt + 1])
                # c = step(s - m) as 0/1 bf16; accum_out gets the row count
                # (op1 is the REDUCTION op when accum_out is present)
                c_sb = work.tile([128, NL], bf16, tag="c")
                nc.vector.tensor_scalar(c_sb[:], u_sb[:], 0.0, None,
                                        op0=alu.is_gt, op1=alu.add,
                                        accum_out=accc[:, jt:jt + 1])
                z_sb = work.tile([128, NL], bf16, tag="z")
                nc.vector.tensor_tensor(z_sb[:], a_sb[:], u_sb[:],
                                        op=alu.subtract)

                pending = (jt, z_sb, c_sb)

            emit_proj(*pending)

            # -- final reduction ------------------------------------------
            out_sb = acc_pool.tile([128, 4], f32)
            nc.vector.reduce_sum(out_sb[:, 0:1], accu[:],
                                 axis=mybir.AxisListType.X)
            nc.vector.reduce_sum(out_sb[:, 1:2], accc[:],
                                 axis=mybir.AxisListType.X)
            junk = acc_pool.tile([128, NL], f32)
            nc.vector.tensor_tensor(junk[:], psum_projz[:], plt_sb[:],
                                    op=alu.mult)
            nc.vector.reduce_sum(out_sb[:, 2:3], junk[:],
                                 axis=mybir.AxisListType.X)
            nc.vector.memset(out_sb[:, 3:4], 0.0)
            nc.sync.dma_start(out_d[:], out_sb[:])

    nc.compile()
    return nc


def kernel(inputs, targets):
    global _COMPILED, LAST_RESULTS
    from concourse.bass_utils import run_bass_kernel_spmd

    X = np.asarray(inputs, dtype=np.float32)
    t = np.asarray(targets).astype(np.int64)
    assert X.shape == (N, D) and t.shape == (N,)

    XT = np.ascontiguousarray(X.astype(_BF16).T)            # [512, 8192]
    P = (t[:, None] == np.arange(C)[None, :]).astype(_BF16)  # [8192, 128]
    # -bf16(0.3) * P; 0.30078125 is exact in bf16 so P3 entries are exact
    M3 = np.float32(_BF16(MARGIN))
    P3 = (-M3 * P.astype(np.float32)).astype(_BF16)

    if _COMPILED is None:
        _COMPILED = _build()
    nc = _COMPILED

    in_maps = []
    for r in range(NCORES):
        sl = slice(r * NL, (r + 1) * NL)
        in_maps.append({
            "xt": XT,
            "xt_loc": np.ascontiguousarray(XT[:, sl]),
            "p": P,
            "p3": P3,
            "ploc_t": np.ascontiguousarray(P[sl].T),
        })

    res = run_bass_kernel_spmd(nc, in_maps, list(range(NCORES)))
    LAST_RESULTS = res

    # out cols: [sum(u), count(c), sum_pos(a-u) - bf16(m)*count_pos(c), 0]
    # total = sum(u) + m*count(c) + col2
    m64 = np.float64(np.float32(MARGIN))
    total = np.float64(0.0)
    for r in range(NCORES):
        cols = res.results[r]["out"].astype(np.float64).sum(axis=0)
        total += cols[0] + m64 * cols[1] + cols[2]
    return np.asarray(total / N, dtype=np.float32)



# revision 9
# speedup vs baseline: 1.8558x; 1.8558x over previous
"""Contrastive-loss kernel for 8 Trainium2 NeuronCores — v2.

Math (reference):
    sim = X @ X.T; pos = targets outer-equal
    loss = ( sum(pos & sim<1 -> 1-sim) + sum(~pos & sim>m -> sim) ) / n

v2 strategy (vs v1's full-matrix data-parallel):
  * Host sorts rows by class label -> positive pairs live in a narrow
    diagonal band (max class size <= 128 asserted).
  * Symmetry: each 128-column i-block b computes only the cyclic window
    j in [c0-4096, c0+128) (c0 = 128*b).  Unordered block pairs at cyclic
    distance 1..31 are computed once (weight 2), distance 32 twice
    (weight 1 each, "phase 0"), the diagonal block once (weight 1 via
    masks).  ~53% of the baseline matmul volume.
  * Work split: core r owns i-blocks {4r..4r+3} and {60-4r..63-4r}
    (stripes r and 15-r) -> every core: 8 blocks x 4224-wide windows,
    a fully uniform SPMD program.  Host pre-rotates X.T per stripe so
    all windows are compile-time slices.
  * Per 2048-wide PSUM tile: ACT does relu(s-m) with accum_out (row sums
    of u), DVE does (s > m) count with accum_out.  The label structure
    enters ONLY through the last 256 columns of each block's window
    ("band"): three tensor_tensor_reduce ops against host-built masks
    mf (f_neg correction) and mp (f_pos weights) fix up positive pairs
    and the diagonal exactly.
  * Final reduction on host from per-core [128, 72] accumulator columns.
"""

import numpy as np
import ml_dtypes

N = 8192
D = 512
C = 128
NCORES = 8
KT = 4            # k tiles (contraction 512 = 4 x 128)
NB = 8            # i-blocks per core
WIN = 4224        # per-block j window (128 phase0 + 2 x 2048 main)
XW = 4608         # rotated xt width per stripe (windows + stationaries)
MARGIN = 0.3
MMW = 512         # moving width per matmul (PSUM bank limit)

_BF16 = ml_dtypes.bfloat16

_COMPILED = None
LAST_RESULTS = None

# accumulator column maps
#   accA (ACT, [128, 24] f32): 0 = phase0 u (w1); 1+2k,2+2k = T1u,T2u (w2)
#   accD (DVE, [128, 48] f32): 0 = phase0 count (w1);
#       1+5k..5+5k = T1c, T2c, mf*u, m*mf*c, mp*a  for block k
NA = 24
ND = 48


def _build():
    import concourse.tile as tile
    from concourse import bacc, mybir

    nc = bacc.Bacc("TRN2", target_bir_lowering=False, debug=False,
                   num_devices=NCORES)
    bf16 = mybir.dt.bfloat16
    f32 = mybir.dt.float32
    relu = mybir.ActivationFunctionType.Relu
    alu = mybir.AluOpType

    xta_d = nc.dram_tensor("xta", [D, XW], bf16, kind="ExternalInput").ap()
    xtb_d = nc.dram_tensor("xtb", [D, XW], bf16, kind="ExternalInput").ap()
    mf_d = nc.dram_tensor("mf", [128, NB * 256], bf16, kind="ExternalInput").ap()
    mp_d = nc.dram_tensor("mp", [128, NB * 256], bf16, kind="ExternalInput").ap()
    out_d = nc.dram_tensor("out", [128, NA + ND], f32, kind="ExternalOutput").ap()

    with tile.TileContext(nc) as tc:
        with (
            tc.tile_pool(name="xt", bufs=1) as xt_pool,
            tc.tile_pool(name="msk", bufs=1) as msk_pool,
            tc.tile_pool(name="acc", bufs=1) as acc_pool,
            tc.tile_pool(name="u", bufs=3) as u_pool,
            tc.tile_pool(name="c", bufs=3) as c_pool,
            tc.tile_pool(name="band", bufs=2) as band_pool,
            tc.tile_pool(name="ps_main", bufs=2, space="PSUM") as ps_main,
        ):
            xta_sb = [xt_pool.tile([128, XW], bf16, name=f"xta{k}",
                                   tag=f"xta{k}") for k in range(KT)]
            xtb_sb = [xt_pool.tile([128, XW], bf16, name=f"xtb{k}",
                                   tag=f"xtb{k}") for k in range(KT)]

            def load(sb, dram, lo, hi):
                for kt in range(KT):
                    nc.sync.dma_start(sb[kt][:, lo:hi],
                                      dram[kt * 128:(kt + 1) * 128, lo:hi])

            # stationaries + phase0 columns first so PE can start ASAP
            load(xta_sb, xta_d, 4096, XW)
            load(xta_sb, xta_d, 0, 512)
            load(xtb_sb, xtb_d, 4096, XW)
            load(xtb_sb, xtb_d, 0, 512)

            mf_sb = msk_pool.tile([128, NB, 256], bf16)
            mp_sb = msk_pool.tile([128, NB, 256], bf16)
            nc.sync.dma_start(mf_sb[:], mf_d.rearrange("p (b w) -> p b w", b=NB))
            nc.sync.dma_start(mp_sb[:], mp_d.rearrange("p (b w) -> p b w", b=NB))

            # remaining moving columns, block-0-first order
            load(xta_sb, xta_d, 512, 2176)
            load(xta_sb, xta_d, 2176, 4096)
            load(xtb_sb, xtb_d, 512, 2176)
            load(xtb_sb, xtb_d, 2176, 4096)

            accA = acc_pool.tile([128, NA], f32)
            accD = acc_pool.tile([128, ND], f32)
            bias_m = acc_pool.tile([128, 1], f32)
            nc.vector.memset(accA[:], 0.0)
            nc.vector.memset(accD[:], 0.0)
            nc.vector.memset(bias_m[:], -MARGIN)

            def src_of(kb):
                return (xta_sb if kb < 4 else xtb_sb), (kb % 4)

            # ---- phase 0: distance-32 blocks (weight 1), PE warmup ----
            ps0 = ps_main.tile([128, 1024], f32, name="ps0", tag="ps")
            for kb in range(NB):
                sb, kk = src_of(kb)
                st = 4096 + 128 * kk
                for kt in range(KT):
                    nc.tensor.matmul(
                        ps0[:, 128 * kb:128 * (kb + 1)],
                        lhsT=sb[kt][:, st:st + 128],
                        rhs=sb[kt][:, 128 * kk:128 * kk + 128],
                        start=(kt == 0), stop=(kt == KT - 1),
                    )
            u0 = u_pool.tile([128, 1024], bf16, tag="u")
            nc.scalar.activation(u0[:], ps0[:], relu,
                                 bias=bias_m[:], scale=1.0,
                                 accum_out=accA[:, 0:1])
            c0 = c_pool.tile([128, 1024], bf16, tag="c")
            nc.vector.tensor_scalar(c0[:], ps0[:], MARGIN, None,
                                    op0=alu.is_gt, op1=alu.add,
                                    accum_out=accD[:, 0:1])

            # ---- main: per block, two 2048-wide psum tiles ----
            for kb in range(NB):
                sb, kk = src_of(kb)
                st = 4096 + 128 * kk
                for half in range(2):
                    ps = ps_main.tile([128, 2048], f32, tag="ps")
                    w0 = 128 + 2048 * half + 128 * kk
                    for cs in range(2048 // MMW):
                        for kt in range(KT):
                            nc.tensor.matmul(
                                ps[:, MMW * cs:MMW * (cs + 1)],
                                lhsT=sb[kt][:, st:st + 128],
                                rhs=sb[kt][:, w0 + MMW * cs:w0 + MMW * (cs + 1)],
                                start=(kt == 0), stop=(kt == KT - 1),
                            )
                    ut = u_pool.tile([128, 2048], bf16, tag="u")
                    nc.scalar.activation(ut[:], ps[:], relu,
                                         bias=bias_m[:], scale=1.0,
                                         accum_out=accA[:, 1 + 2 * kb + half:
                                                        2 + 2 * kb + half])
                    ct = c_pool.tile([128, 2048], bf16, tag="c")
                    nc.vector.tensor_scalar(ct[:], ps[:], MARGIN, None,
                                            op0=alu.is_gt, op1=alu.add,
                                            accum_out=accD[:, 1 + 5 * kb + half:
                                                           2 + 5 * kb + half])
                    if half == 1:
                        at = band_pool.tile([128, 256], bf16, tag="a")
                        nc.scalar.activation(at[:], ps[:, 1792:2048], relu,
                                             bias=1.0, scale=-1.0)
                        # mf uses MARGIN-scaled c: fold m into the mask on
                        # the c term by accumulating count*mf separately
                        s1 = band_pool.tile([128, 256], bf16, tag="s1")
                        nc.vector.tensor_tensor(s1[:], ut[:, 1792:2048],
                                                mf_sb[:, kb, :], op=alu.mult)
                        nc.vector.tensor_reduce(
                            accD[:, 3 + 5 * kb:4 + 5 * kb], s1[:],
                            axis=mybir.AxisListType.X, op=alu.add)
                        s2 = band_pool.tile([128, 256], bf16, tag="s2")
                        nc.vector.tensor_tensor(s2[:], ct[:, 1792:2048],
                                                mf_sb[:, kb, :], op=alu.mult)
                        nc.vector.tensor_reduce(
                            accD[:, 4 + 5 * kb:5 + 5 * kb], s2[:],
                            axis=mybir.AxisListType.X, op=alu.add)
                        s3 = band_pool.tile([128, 256], bf16, tag="s3")
                        nc.vector.tensor_tensor(s3[:], at[:],
                                                mp_sb[:, kb, :], op=alu.mult)
                        nc.vector.tensor_reduce(
                            accD[:, 5 + 5 * kb:6 + 5 * kb], s3[:],
                            axis=mybir.AxisListType.X, op=alu.add)

            nc.sync.dma_start(out_d[:, 0:NA], accA[:])
            nc.sync.dma_start(out_d[:, NA:NA + ND], accD[:])

    nc.compile()
    return nc


def kernel(inputs, targets):
    global _COMPILED, LAST_RESULTS
    from concourse.bass_utils import run_bass_kernel_spmd

    X = np.asarray(inputs, dtype=np.float32)
    t = np.asarray(targets).astype(np.int64)
    assert X.shape == (N, D) and t.shape == (N,)

    order = np.argsort(t, kind="stable")
    ts = t[order]
    assert np.bincount(ts, minlength=C).max() <= 128, "class straddle > 128"
    XT = np.ascontiguousarray(X[order].astype(_BF16).T)      # [512, 8192]

    if _COMPILED is None:
        _COMPILED = _build()
    nc = _COMPILED

    in_maps = []
    for r in range(NCORES):
        m = {}
        for name, sblk in (("xta", 4 * r), ("xtb", 60 - 4 * r)):
            base = 128 * sblk              # global col of the stripe start
            rot = (base - 4096) % N        # rotated col 0 = this global col
            idx = (rot + np.arange(XW)) % N
            m[name] = np.ascontiguousarray(XT[:, idx])
        mf = np.zeros((128, NB, 256), dtype=_BF16)
        mp = np.zeros((128, NB, 256), dtype=_BF16)
        blocks = [4 * r + k for k in range(4)] + \
                 [60 - 4 * r + k for k in range(4)]
        for kb, b in enumerate(blocks):
            c0 = 128 * b
            jb = (np.arange(c0 - 128, c0 + 128)) % N
            same = ts[c0:c0 + 128][:, None] == ts[jb][None, :]
            w = np.where(np.arange(256) < 128, 2.0, 1.0)[None, :]
            mf[:, kb, :] = np.where(same, -2.0, w - 2.0).astype(_BF16)
            mp[:, kb, :] = np.where(same, w, 0.0).astype(_BF16)
        m["mf"] = np.ascontiguousarray(mf.reshape(128, NB * 256))
        m["mp"] = np.ascontiguousarray(mp.reshape(128, NB * 256))
        in_maps.append(m)

    res = run_bass_kernel_spmd(nc, in_maps, list(range(NCORES)))
    LAST_RESULTS = res

    m64 = np.float64(np.float32(MARGIN))
    total = np.float64(0.0)
    for r in range(NCORES):
        o = res.results[r]["out"].astype(np.float64).sum(axis=0)
        A, Dv = o[:NA], o[NA:]
        total += A[0] + m64 * Dv[0]                      # phase0, w1
        for kb in range(NB):
            total += 2.0 * (A[1 + 2 * kb] + A[2 + 2 * kb])
            total += 2.0 * m64 * (Dv[1 + 5 * kb] + Dv[2 + 5 * kb])
            total += Dv[3 + 5 * kb] + m64 * Dv[4 + 5 * kb] + Dv[5 + 5 * kb]
    return np.asarray(total / N, dtype=np.float32)


# revision 10
# speedup vs baseline: 1.8999x; 1.0238x over previous
"""Contrastive-loss kernel for 8 Trainium2 NeuronCores — v2.

Math (reference):
    sim = X @ X.T; pos = targets outer-equal
    loss = ( sum(pos & sim<1 -> 1-sim) + sum(~pos & sim>m -> sim) ) / n

v2 strategy (vs v1's full-matrix data-parallel):
  * Host sorts rows by class label -> positive pairs live in a narrow
    diagonal band (max class size <= 128 asserted).
  * Symmetry: each 128-column i-block b computes only the cyclic window
    j in [c0-4096, c0+128) (c0 = 128*b).  Unordered block pairs at cyclic
    distance 1..31 are computed once (weight 2), distance 32 twice
    (weight 1 each, "phase 0"), the diagonal block once (weight 1 via
    masks).  ~53% of the baseline matmul volume.
  * Work split: core r owns i-blocks {4r..4r+3} and {60-4r..63-4r}
    (stripes r and 15-r) -> every core: 8 blocks x 4224-wide windows,
    a fully uniform SPMD program.  Host pre-rotates X.T per stripe so
    all windows are compile-time slices.
  * Per 2048-wide PSUM tile: ACT does relu(s-m) with accum_out (row sums
    of u), DVE does (s > m) count with accum_out.  The label structure
    enters ONLY through the last 256 columns of each block's window
    ("band"): three tensor_tensor_reduce ops against host-built masks
    mf (f_neg correction) and mp (f_pos weights) fix up positive pairs
    and the diagonal exactly.
  * Final reduction on host from per-core [128, 72] accumulator columns.
"""

import numpy as np
import ml_dtypes

N = 8192
D = 512
C = 128
NCORES = 8
KT = 4            # k tiles (contraction 512 = 4 x 128)
NB = 8            # i-blocks per core
WIN = 4224        # per-block j window (128 phase0 + 2 x 2048 main)
XW = 4608         # rotated xt width per stripe (windows + stationaries)
MARGIN = 0.3
MMW = 512         # moving width per matmul (PSUM bank limit)

_BF16 = ml_dtypes.bfloat16

_COMPILED = None
LAST_RESULTS = None

# accumulator column maps
#   accA (ACT, [128, 24] f32): 0 = phase0 u (w1); 1+2k,2+2k = T1u,T2u (w2)
#   accD (DVE, [128, 48] f32): 0 = phase0 count (w1);
#       1+5k..5+5k = T1c, T2c, mf*u, m*mf*c, mp*a  for block k
NA = 24
ND = 48


def _build():
    import concourse.tile as tile
    from concourse import bacc, mybir

    nc = bacc.Bacc("TRN2", target_bir_lowering=False, debug=False,
                   num_devices=NCORES)
    bf16 = mybir.dt.bfloat16
    f32 = mybir.dt.float32
    relu = mybir.ActivationFunctionType.Relu
    alu = mybir.AluOpType

    xta_d = nc.dram_tensor("xta", [D, XW], bf16, kind="ExternalInput").ap()
    xtb_d = nc.dram_tensor("xtb", [D, XW], bf16, kind="ExternalInput").ap()
    mf_d = nc.dram_tensor("mf", [128, NB * 256], bf16, kind="ExternalInput").ap()
    mp_d = nc.dram_tensor("mp", [128, NB * 256], bf16, kind="ExternalInput").ap()
    out_d = nc.dram_tensor("out", [128, NA + ND], f32, kind="ExternalOutput").ap()

    with tile.TileContext(nc) as tc:
        with (
            tc.tile_pool(name="xt", bufs=1) as xt_pool,
            tc.tile_pool(name="msk", bufs=1) as msk_pool,
            tc.tile_pool(name="acc", bufs=1) as acc_pool,
            tc.tile_pool(name="u", bufs=3) as u_pool,
            tc.tile_pool(name="c", bufs=3) as c_pool,
            tc.tile_pool(name="band", bufs=2) as band_pool,
            tc.tile_pool(name="ps_main", bufs=2, space="PSUM") as ps_main,
        ):
            xta_sb = [xt_pool.tile([128, XW], bf16, name=f"xta{k}",
                                   tag=f"xta{k}") for k in range(KT)]
            xtb_sb = [xt_pool.tile([128, XW], bf16, name=f"xtb{k}",
                                   tag=f"xtb{k}") for k in range(KT)]

            def load(sb, dram, lo, hi):
                for kt in range(KT):
                    nc.sync.dma_start(sb[kt][:, lo:hi],
                                      dram[kt * 128:(kt + 1) * 128, lo:hi])

            # stationaries + phase0 columns first so PE can start ASAP
            load(xta_sb, xta_d, 4096, XW)
            load(xta_sb, xta_d, 0, 512)
            load(xtb_sb, xtb_d, 4096, XW)
            load(xtb_sb, xtb_d, 0, 512)

            mf_sb = msk_pool.tile([128, NB, 256], bf16)
            mp_sb = msk_pool.tile([128, NB, 256], bf16)
            nc.sync.dma_start(mf_sb[:], mf_d.rearrange("p (b w) -> p b w", b=NB))
            nc.sync.dma_start(mp_sb[:], mp_d.rearrange("p (b w) -> p b w", b=NB))

            # remaining moving columns, block-0-first order
            load(xta_sb, xta_d, 512, 2176)
            load(xta_sb, xta_d, 2176, 4096)
            load(xtb_sb, xtb_d, 512, 2176)
            load(xtb_sb, xtb_d, 2176, 4096)

            accA = acc_pool.tile([128, NA], f32)
            accD = acc_pool.tile([128, ND], f32)
            bias_m = acc_pool.tile([128, 1], f32)
            nc.vector.memset(accA[:], 0.0)
            nc.vector.memset(accD[:], 0.0)
            nc.vector.memset(bias_m[:], -MARGIN)

            def src_of(kb):
                return (xta_sb if kb < 4 else xtb_sb), (kb % 4)

            # ---- phase 0: distance-32 blocks (weight 1), PE warmup ----
            ps0 = ps_main.tile([128, 1024], f32, name="ps0", tag="ps")
            for kb in range(NB):
                sb, kk = src_of(kb)
                st = 4096 + 128 * kk
                for kt in range(KT):
                    nc.tensor.matmul(
                        ps0[:, 128 * kb:128 * (kb + 1)],
                        lhsT=sb[kt][:, st:st + 128],
                        rhs=sb[kt][:, 128 * kk:128 * kk + 128],
                        start=(kt == 0), stop=(kt == KT - 1),
                    )
            u0 = u_pool.tile([128, 1024], bf16, tag="u")
            nc.scalar.activation(u0[:], ps0[:], relu,
                                 bias=bias_m[:], scale=1.0,
                                 accum_out=accA[:, 0:1])
            c0 = c_pool.tile([128, 1024], bf16, tag="c")
            nc.vector.tensor_scalar(c0[:], ps0[:], MARGIN, None,
                                    op0=alu.is_gt, op1=alu.add,
                                    accum_out=accD[:, 0:1])

            # ---- main: per block, two 2048-wide psum tiles ----
            for kb in range(NB):
                sb, kk = src_of(kb)
                st = 4096 + 128 * kk
                for half in range(2):
                    ps = ps_main.tile([128, 2048], f32, tag="ps")
                    w0 = 128 + 2048 * half + 128 * kk
                    for kt in range(KT):
                        for cs in range(2048 // MMW):
                            nc.tensor.matmul(
                                ps[:, MMW * cs:MMW * (cs + 1)],
                                lhsT=sb[kt][:, st:st + 128],
                                rhs=sb[kt][:, w0 + MMW * cs:w0 + MMW * (cs + 1)],
                                start=(kt == 0), stop=(kt == KT - 1),
                            )
                    ut = u_pool.tile([128, 2048], bf16, tag="u")
                    nc.scalar.activation(ut[:], ps[:], relu,
                                         bias=bias_m[:], scale=1.0,
                                         accum_out=accA[:, 1 + 2 * kb + half:
                                                        2 + 2 * kb + half])
                    ct = c_pool.tile([128, 2048], bf16, tag="c")
                    nc.vector.tensor_scalar(ct[:], ps[:], MARGIN, None,
                                            op0=alu.is_gt, op1=alu.add,
                                            accum_out=accD[:, 1 + 5 * kb + half:
                                                           2 + 5 * kb + half])
                    if half == 1:
                        at = band_pool.tile([128, 256], bf16, tag="a")
                        nc.scalar.activation(at[:], ps[:, 1792:2048], relu,
                                             bias=1.0, scale=-1.0)
                        # mf uses MARGIN-scaled c: fold m into the mask on
                        # the c term by accumulating count*mf separately
                        s1 = band_pool.tile([128, 256], bf16, tag="s1")
                        nc.vector.tensor_tensor(s1[:], ut[:, 1792:2048],
                                                mf_sb[:, kb, :], op=alu.mult)
                        nc.vector.tensor_reduce(
                            accD[:, 3 + 5 * kb:4 + 5 * kb], s1[:],
                            axis=mybir.AxisListType.X, op=alu.add)
                        s2 = band_pool.tile([128, 256], bf16, tag="s2")
                        nc.vector.tensor_tensor(s2[:], ct[:, 1792:2048],
                                                mf_sb[:, kb, :], op=alu.mult)
                        nc.vector.tensor_reduce(
                            accD[:, 4 + 5 * kb:5 + 5 * kb], s2[:],
                            axis=mybir.AxisListType.X, op=alu.add)
                        s3 = band_pool.tile([128, 256], bf16, tag="s3")
                        nc.vector.tensor_tensor(s3[:], at[:],
                                                mp_sb[:, kb, :], op=alu.mult)
                        nc.vector.tensor_reduce(
                            accD[:, 5 + 5 * kb:6 + 5 * kb], s3[:],
                            axis=mybir.AxisListType.X, op=alu.add)

            nc.sync.dma_start(out_d[:, 0:NA], accA[:])
            nc.sync.dma_start(out_d[:, NA:NA + ND], accD[:])

    nc.compile()
    return nc


def kernel(inputs, targets):
    global _COMPILED, LAST_RESULTS
    from concourse.bass_utils import run_bass_kernel_spmd

    X = np.asarray(inputs, dtype=np.float32)
    t = np.asarray(targets).astype(np.int64)
    assert X.shape == (N, D) and t.shape == (N,)

    order = np.argsort(t, kind="stable")
    ts = t[order]
    assert np.bincount(ts, minlength=C).max() <= 128, "class straddle > 128"
    XT = np.ascontiguousarray(X[order].astype(_BF16).T)      # [512, 8192]

    if _COMPILED is None:
        _COMPILED = _build()
    nc = _COMPILED

    in_maps = []
    for r in range(NCORES):
        m = {}
        for name, sblk in (("xta", 4 * r), ("xtb", 60 - 4 * r)):
            base = 128 * sblk              # global col of the stripe start
            rot = (base - 4096) % N        # rotated col 0 = this global col
            idx = (rot + np.arange(XW)) % N
            m[name] = np.ascontiguousarray(XT[:, idx])
        mf = np.zeros((128, NB, 256), dtype=_BF16)
        mp = np.zeros((128, NB, 256), dtype=_BF16)
        blocks = [4 * r + k for k in range(4)] + \
                 [60 - 4 * r + k for k in range(4)]
        for kb, b in enumerate(blocks):
            c0 = 128 * b
            jb = (np.arange(c0 - 128, c0 + 128)) % N
            same = ts[c0:c0 + 128][:, None] == ts[jb][None, :]
            w = np.where(np.arange(256) < 128, 2.0, 1.0)[None, :]
            mf[:, kb, :] = np.where(same, -2.0, w - 2.0).astype(_BF16)
            mp[:, kb, :] = np.where(same, w, 0.0).astype(_BF16)
        m["mf"] = np.ascontiguousarray(mf.reshape(128, NB * 256))
        m["mp"] = np.ascontiguousarray(mp.reshape(128, NB * 256))
        in_maps.append(m)

    res = run_bass_kernel_spmd(nc, in_maps, list(range(NCORES)))
    LAST_RESULTS = res

    m64 = np.float64(np.float32(MARGIN))
    total = np.float64(0.0)
    for r in range(NCORES):
        o = res.results[r]["out"].astype(np.float64).sum(axis=0)
        A, Dv = o[:NA], o[NA:]
        total += A[0] + m64 * Dv[0]                      # phase0, w1
        for kb in range(NB):
            total += 2.0 * (A[1 + 2 * kb] + A[2 + 2 * kb])
            total += 2.0 * m64 * (Dv[1 + 5 * kb] + Dv[2 + 5 * kb])
            total += Dv[3 + 5 * kb] + m64 * Dv[4 + 5 * kb] + Dv[5 + 5 * kb]
    return np.asarray(total / N, dtype=np.float32)
